# revision 36
# baseline (speedup 1.0000x reference)
"""CameraMemory circle-loss kernel for 8 Trainium2 NeuronCores.

Strategy (fp8 features-stationary, 128-partition dense)
-------------------------------------------------------
reference computes:
    x        = normalize(inputs)                      [B, D]
    out      = (x @ features.T + 1) / 2               [B, N]
    loss_p   = sum over {pids[j]==targets[b]}  of exp(5*(1-s)^2)
    loss_n   = sum over {pids[j]!=targets[b] and camids[j]==cams[b]}
                                               of exp(5*(1+s)^2)
    return log1p(loss_p * loss_n)        (s = x.f raw cosine)

- loss_n's camera mask is made *block diagonal* by sorting the memory bank
  by camid on the host; rows (batch) are grouped by cam.  The device only
  computes the block-diagonal similarities (1/8th the dense work).
- The tiny pid-matching subset that must be excluded from loss_n, and all
  of loss_p, are computed on host (sparse: ~29k of 25.6M pairs).
- x and features are quantized to fp8e4 (e4m3) * 16 on host.  The induced
  dot error (sigma ~0.0024) perturbs loss_n by <0.1%, far inside the 2e-2
  gate.  fp8 halves HBM traffic vs bf16 and enables the PE DoubleRow perf
  mode (K=256 in one matmul).

Device kernel (identical program on all 8 cores, different data):
  - features are STATIONARY: each matmul computes
        psum[128 featcols, nrows] = fchunk[K].T @ xrows[K]
    so PSUM uses all 128 partitions and the moving free dim is the cam's
    batch-row count (no alignment constraint).  Chunk row-blocks pack
    back-to-back into PSUM banks, splitting at bank boundaries: squares
    and exps run at full 128-partition density with zero gaps.
  - x (fp8, [128, 2, 256]) is loaded ONCE; feature chunks stream in
    bank-aligned DMA tiles at the 360 GB/s cost-model rate.  Bank sizes
    shrink geometrically at the end of the stream so the drain (square,
    exp, result DMA) trails the last bytes by as little as possible.
  - the elementwise exp-of-square is ONE fused ACT op: Derivative_Erf
    with folded scale+bias computes (2/sqrt(pi)) e^{-5(1+s)^2} straight
    from raw PSUM; DVE reciprocal inverts it to e^{+5(1+s)^2} (bf16, the
    per-term noise cancels over 3.2M terms); accumulation is a partition
    reduce on the otherwise-idle GPSIMD (or DVE for the stream-tail unit).
    This splits the element pipeline evenly across ACT/DVE/Pool, all
    hidden under the DMA stream.
  - zero-padded feature columns contribute exactly e^5 per (col,row);
    the host subtracts npad*e5 and applies the sqrt(pi)/2 factor.
"""

import numpy as np

B, D = 256, 256
NCORES = 8
TEMP = 0.05
EPS = 1e-12
CK = 128          # feature columns per chunk (PSUM partition dim)
KT = 2            # K tiles (D = KT * 128), fused by DoubleRow
SLOT = 512        # one PSUM bank (f32 columns)
SC = 16.0         # fp8 quantization scale for x and features
SQRT5 = float(np.sqrt(5.0))
E5 = float(np.exp(np.float64(5.0)))

# ---- schedule knobs (tuned against the TimelineSim cost model) ----
# tail unit sizes after the 1024-wide steady-state units (geometrically
# shrinking so the end-of-stream drain chain is short)
TAIL_CAPS = [384, 288, 96]

_NC_CACHE = {}


def _mdt():
    import concourse.mybir as mybir

    return mybir.dt.float8e4


def _np_fp8():
    import ml_dtypes

    return ml_dtypes.float8_e4m3


def _schedule(rows, kc):
    """Uniform per-core device schedule.

    Columns of all chunks are packed into one global column space with
    physical PSUM-bank boundaries every 512 columns.  "Units" are the
    square/exp instruction granularity (wide early, tiny at the stream
    tail); each unit is also one feature DMA tile.  PSUM is a handful of
    single-use tiles covering the whole space (fits in the 8 banks).
    """
    chunk_cams = []
    for c in range(len(rows)):
        chunk_cams += [c] * kc[c]
    total = sum(rows[c] for c in chunk_cams)

    # units: 512-col (one PSUM bank) steady state; geometric tail
    tail_total = sum(TAIL_CAPS)
    big_total = max(0, total - tail_total)
    units = []
    while big_total >= SLOT:
        units.append(SLOT)
        big_total -= SLOT
    if big_total > 0:
        units.append(big_total)      # leftover big unit right before the tail
    rem = min(total, tail_total)
    for cap in TAIL_CAPS:
        take = min(cap, rem)
        if take > 0:
            units.append(take)
            rem -= take
    nunit = len(units)
    ustart = np.concatenate([[0], np.cumsum(units)]).astype(int)

    # pack segments; split at 512 boundaries and unit boundaries
    bounds = sorted(set(
        [int(x) for x in range(0, total + 1, SLOT)] + [int(x) for x in ustart]
    ))
    segs = []      # (chunk_idx, cam, rowoff, nrows, globaloff)
    pos = 0
    for ci, c in enumerate(chunk_cams):
        nr, ro = rows[c], 0
        while nr > 0:
            nxt = min(b for b in bounds if b > pos)
            a = min(nr, nxt - pos)
            segs.append((ci, c, ro, a, pos))
            pos += a
            ro += a
            nr -= a
    assert pos == total

    # chunk -> bound interval (where the chunk starts); DMA tile per interval
    start_iv = {}
    for ci2, c2, ro2, a2, g2 in segs:
        if ci2 not in start_iv:
            start_iv[ci2] = int(np.searchsorted(bounds, g2, "right") - 1)
    niv = len(bounds) - 1
    tiles = []
    lo = 0
    for iv in range(niv):
        hi = max([ci + 1 for ci in start_iv if start_iv[ci] == iv],
                 default=lo)
        tiles.append((lo, hi))
        lo = max(lo, hi)
    chunk_tile = {}
    for t, (clo, chi) in enumerate(tiles):
        for ci2 in range(clo, chi):
            chunk_tile[ci2] = t

    # one pass per unit; the last two passes reduce on DVE (same engine as
    # the reciprocal, no cross-engine hop), earlier ones on Pool
    pool_acc = [u != nunit - 2 for u in range(nunit)]

    # physical PSUM tiles: one per unit (shared tiles would false-serialize
    # the next unit's matmuls behind this unit's activation read)
    ptiles = list(units)
    pstart = np.concatenate([[0], np.cumsum(ptiles)]).astype(int)

    return {
        "chunk_cams": chunk_cams,
        "segs": segs,
        "units": units,
        "ptiles": ptiles,
        "pstart": pstart,
        "ustart": ustart,
        "tiles": tiles,
        "chunk_tile": chunk_tile,
        "pool_acc": pool_acc,
        "nchunk": len(chunk_cams),
        "total": total,
    }


def _build_bass(rows, kc):
    import concourse.bacc as bacc
    import concourse.mybir as mybir
    import concourse.tile as tile

    dt = mybir.dt
    AF = mybir.ActivationFunctionType
    mdt = _mdt()

    sch = _schedule(rows, kc)
    segs, units, ustart = sch["segs"], sch["units"], sch["ustart"]
    tiles, chunk_tile = sch["tiles"], sch["chunk_tile"]
    pool_acc, nchunk = sch["pool_acc"], sch["nchunk"]
    nunit = len(units)
    npass = nunit
    xoff = np.concatenate([[0], np.cumsum(rows)]).astype(int)

    # segments grouped by unit
    usegs = [[] for _ in range(nunit)]
    for ci, c, ro, a, g in segs:
        u = int(np.searchsorted(ustart, g, "right") - 1)
        usegs[u].append((ci, c, ro, a, g - ustart[u]))

    nc = bacc.Bacc("TRN2", target_bir_lowering=False)
    fl = nc.dram_tensor("fl", [128, nchunk * KT * CK], mdt, kind="ExternalInput")
    xq = nc.dram_tensor("xq", [128, KT, B], mdt, kind="ExternalInput")
    out = nc.dram_tensor("out", [128, npass], dt.float32, kind="ExternalOutput")

    with tile.TileContext(nc) as tc:
        with (
            tc.tile_pool(name="fpool", bufs=3) as fpool,
            tc.tile_pool(name="psum", bufs=1, space="PSUM") as pspool,
            tc.tile_pool(name="dpool", bufs=2) as dpool,
            tc.tile_pool(name="rpool2", bufs=2) as rcpool,
            tc.tile_pool(name="res", bufs=1) as rpool,
        ):
            allparts = rpool.tile([128, npass], dt.float32)
            bias_t = rpool.tile([128, 1], dt.float32)
            nc.vector.memset(bias_t, SQRT5)
            xt = rpool.tile([128, KT, B], mdt)
            # prime the ACT table before the steady-state loop
            sc0 = rpool.tile([128, 1], dt.float32)
            nc.vector.memset(sc0, 0.0)
            sc1 = rpool.tile([128, 1], dt.float32)
            nc.scalar.activation(sc1, sc0, AF.Derivative_Erf)

            ftiles = {}
            first_issue = [True]

            def want_tile(t):
                if t < len(tiles) and t not in ftiles and tiles[t][1] > tiles[t][0]:
                    clo, chi = tiles[t]
                    ft = fpool.tile(
                        [128, (chi - clo), KT, CK], mdt,
                        tag=f"f{chi - clo}", name=f"ft{t}",
                    )
                    nc.sync.dma_start(
                        out=ft, in_=fl[:, clo * KT * CK : chi * KT * CK]
                    )
                    ftiles[t] = ft
                    if first_issue[0]:
                        first_issue[0] = False
                        nc.sync.dma_start(out=xt, in_=xq[:, :, :])

            for u in range(nunit):
                w = units[u]
                ps = pspool.tile([128, w], dt.float32, tag=f"psu{u}", name=f"ps{u}")
                for ci, c, ro, a, off in usegs[u]:
                    t = chunk_tile[ci]
                    want_tile(t)
                    want_tile(t + 1)
                    g = ci - tiles[t][0]
                    nc.tensor.matmul(
                        ps[:, off : off + a],
                        lhsT=ftiles[t][:, g],
                        rhs=xt[:, :, xoff[c] + ro : xoff[c] + ro + a],
                        start=True,
                        stop=True,
                        perf_mode=mybir.MatmulPerfMode.DoubleRow,
                    )
                # derf(ps * sqrt5/SC^2 + sqrt5) = (2/sqrt(pi)) e^{-5(1+s)^2}
                dv = dpool.tile([128, w], dt.bfloat16, tag=f"dv{w}", name=f"dv{u}")
                nc.scalar.activation(
                    dv, ps, AF.Derivative_Erf, bias=bias_t, scale=SQRT5 / (SC * SC)
                )
                rc = rcpool.tile([128, w], dt.bfloat16, tag=f"rc{w}", name=f"rc{u}")
                with nc.allow_low_precision(reason="terms summed over 3e6; bf16 noise cancels"):
                    nc.vector.reciprocal(rc, dv)
                if pool_acc[u]:
                    nc.gpsimd.tensor_reduce(
                        out=allparts[0:1, u : u + 1],
                        in_=rc,
                        axis=mybir.AxisListType.XYZWC,
                        op=mybir.AluOpType.add,
                    )
                else:
                    nc.vector.tensor_reduce(
                        out=allparts[:, u : u + 1],
                        in_=rc,
                        axis=mybir.AxisListType.XYZW,
                        op=mybir.AluOpType.add,
                    )
            nc.sync.dma_start(out=out[:, :], in_=allparts)
    nc.compile()
    return nc, sch


def _host_sparse_sums(x, features, targets, cams, pids, camids):
    """loss_p (all pid-matching pairs) and J (pid AND cam matching pairs),
    mirroring the reference formulas, summed in float64."""
    loss_p = 0.0
    jsum = 0.0
    order_p = np.argsort(pids, kind="stable")
    pids_sorted = pids[order_p]
    for t in np.unique(targets):
        rows = np.flatnonzero(targets == t)
        lo = np.searchsorted(pids_sorted, t, "left")
        hi = np.searchsorted(pids_sorted, t, "right")
        js = order_p[lo:hi]
        if len(js) == 0 or len(rows) == 0:
            continue
        sub = x[rows] @ features[js].T
        o = ((sub + np.float32(1.0)) * np.float32(0.5)).astype(np.float32)
        ap = np.maximum(np.float32(1.0) - o, np.float32(0.0))
        termp = np.exp(-ap * (o - np.float32(1.0)) / np.float32(TEMP))
        loss_p += termp.sum(dtype=np.float64)
        cam_eq = camids[js][None, :] == cams[rows][:, None]
        if cam_eq.any():
            an = np.maximum(o, np.float32(0.0))
            termn = np.exp(an * o / np.float32(TEMP))
            jsum += termn[cam_eq].sum(dtype=np.float64)
    return loss_p, jsum


def _prepare(inputs):
    """Host-side prep: normalize, sparse sums, quantize+pack device data,
    build+compile the bass module."""
    np_fp8 = _np_fp8()
    x_in = np.ascontiguousarray(np.asarray(inputs["inputs"], dtype=np.float32))
    features = np.ascontiguousarray(np.asarray(inputs["features"], dtype=np.float32))
    targets = np.asarray(inputs["targets"]).astype(np.int64)
    cams = np.asarray(inputs["cams"]).astype(np.int64)
    pids = np.asarray(inputs["pids"]).astype(np.int64)
    camids = np.asarray(inputs["camids"]).astype(np.int64)

    nrm = np.sqrt(np.sum(x_in * x_in, axis=1, keepdims=True, dtype=np.float32))
    x = x_in / np.maximum(nrm, np.float32(EPS))

    loss_p, jsum = _host_sparse_sums(x, features, targets, cams, pids, camids)

    # ---- cam-sorted layout ----
    ncam = int(max(cams.max(), camids.max())) + 1
    perm = np.argsort(camids, kind="stable")
    feat_s = features[perm]                                  # [N, D]
    ncols = np.bincount(camids, minlength=ncam).astype(int)
    colstart = np.concatenate([[0], np.cumsum(ncols)]).astype(int)
    rowperm = np.argsort(cams, kind="stable")
    rows = np.bincount(cams, minlength=ncam).astype(int)
    assert rows.max() <= SLOT and rows.sum() == B

    # ---- fp8 quantization ----
    fq = (SC * feat_s).astype(np_fp8)                        # [N, D]
    xq_r = (SC * x[rowperm]).astype(np_fp8)                  # [B, D]

    gc = [(ncols[c] + CK - 1) // CK for c in range(ncam)]    # global chunks
    kc = [(g + NCORES - 1) // NCORES for g in gc]            # per-core chunks

    key = (tuple(int(r) for r in rows), tuple(kc))
    if key not in _NC_CACHE:
        _NC_CACHE[key] = _build_bass([int(r) for r in rows], kc)
    nc, sch = _NC_CACHE[key]
    nchunk = sch["nchunk"]

    # chunk j of cam c on core m covers sorted columns
    #   [colstart[c] + (j*8+m)*CK, +CK)  (clipped; zero-padded)
    fl = np.zeros((NCORES, 128, nchunk, KT, CK), dtype=np_fp8)
    realcols = np.zeros((NCORES, nchunk), dtype=np.int64)
    ci = 0
    for c in range(ncam):
        for j in range(kc[c]):
            for m in range(NCORES):
                s0 = (j * NCORES + m) * CK
                w = int(np.clip(ncols[c] - s0, 0, CK))
                realcols[m, ci] = w
                if w > 0:
                    blk = fq[colstart[c] + s0 : colstart[c] + s0 + w]  # [w, D]
                    fl[m, :, ci, :, :w] = blk.reshape(w, KT, 128).transpose(2, 1, 0)
            ci += 1
    fl_dev = np.ascontiguousarray(fl.reshape(NCORES, 128, nchunk * KT * CK))
    xarr = np.ascontiguousarray(xq_r.reshape(B, KT, 128).transpose(2, 1, 0))

    # pad terms per (core, pass): each seg (ci,c,ro,a,off) contributes
    # (CK - realcols[m, ci]) * a zero columns -> e^5 each
    ustart = sch["ustart"]
    npad = np.zeros((NCORES, len(sch["units"])), dtype=np.float64)
    for ci2, c2, ro2, a2, g2 in sch["segs"]:
        u = int(np.searchsorted(ustart, g2, "right") - 1)
        npad[:, u] += (CK - realcols[:, ci2]) * a2

    return {
        "nc": nc,
        "in_maps": [{"fl": fl_dev[m], "xq": xarr} for m in range(NCORES)],
        "npad": npad,
        "pool_acc": sch["pool_acc"],
        "loss_p": loss_p,
        "jsum": jsum,
    }


def _reduce(prep, results):
    """Combine per-core device partials with the host-side sparse sums."""
    npad = prep["npad"]
    pool_acc = prep["pool_acc"]
    scale = 2.0 / np.sqrt(np.pi)
    loss_n_dev = 0.0
    for m in range(NCORES):
        o = results[m]["out"].astype(np.float64)             # [128, npass]
        for p in range(npad.shape[1]):
            tot = o[0, p] if pool_acc[p] else o[:, p].sum()
            loss_n_dev += tot * scale - npad[m][p] * E5
    loss_n = loss_n_dev - prep["jsum"]
    lp = np.float64(np.float32(prep["loss_p"]))
    ln = np.float64(np.float32(loss_n))
    return np.float32(np.log1p(lp * ln))


def kernel(**inputs):
    prep = _prepare(inputs)
    from concourse.bass_utils import run_bass_kernel_spmd

    res = run_bass_kernel_spmd(
        prep["nc"], prep["in_maps"], core_ids=list(range(NCORES))
    )
    return _reduce(prep, res.results)
